# revision 23
# baseline (speedup 1.0000x reference)
"""Causal self-attention (B=2, T=2048, C=1024, H=16, D=64) on 8 NeuronCores.

Sharding: core = (batch b, head-group g); each of the 8 cores handles one
batch and 4 of the 16 heads (data parallel on B, tensor parallel on heads).
Each core computes q/k/v projections for its heads, rope, causal softmax
attention, and a partial out-projection; the host sums the 4 per-batch
partials and adds bout + bv@Wout (the V-bias commutes through the softmax
because the attention weights sum to 1).

Device dataflow (per core), matmul operands in bf16 (PSUM accumulates fp32;
bf16 halves DMA, enables fast weight loads, and keeps the PE at full clock):
  - host passes x[b].T in bf16 so the contraction dim (C) lands on partitions
  - q,k are produced directly in [dim, t] layout ("A"=low rotary halves of
    4 heads stacked, "B"=high halves); bias-add on DVE, rope on DVE/GpSimd
    with fp32 intermediates
  - S^T = K~^T Q~ per 128k x 512q block, 4 heads packed into the PE array
    via tile_position row groups (contraction=32 each for A/B parts)
  - softmax without max-subtraction (logits are O(1) for this model):
    exp on ScalarE with the 1/8 scale folded in, bf16 output; causal
    masking by multiplying diagonal blocks with constant 0/1 masks
  - O^T = V_aug^T expS^T accumulated over k blocks, where V_aug carries a
    ones column so row 64 of the PSUM accumulator is the softmax
    denominator; divide via DVE reciprocal + K=1 PE broadcast of the recip
    (the DVE multiply reads the broadcast straight from PSUM)
  - partial out-projection [t,c] = (O^T)^T Wout_rows, DMA'd out in fp32
"""
import sys
sys.path.insert(0, '/opt/trn_rl_repo')

import numpy as np
import ml_dtypes
from contextlib import ExitStack

import concourse.bass as bass
import concourse.tile as tile
from concourse import mybir
from concourse.bass_utils import run_bass_kernel_spmd

B, T, C, H, D = 2, 2048, 1024, 16, 64
HPC = 4          # heads per core
G = H // HPC     # head groups (cores per batch)
N_CORES = B * G
SCALE = 1.0 / np.sqrt(D)
P = 128
QT = 512         # q tile width
TT = T // QT     # q tiles
NKB = T // P     # 128-wide k blocks
NTB = T // P     # 128-wide t blocks
NCC = C // P     # 128-deep contraction chunks
F32 = mybir.dt.float32
F32R = mybir.dt.float32r
BF16 = mybir.dt.bfloat16
BF = ml_dtypes.bfloat16


def _tril_mask():
    p = np.arange(P)[:, None]
    f = np.arange(P)[None, :]
    return (p <= f).astype(np.float32)


# walrus in this toolchain can't encode >1 sem wait on one instruction
# ("Too many sync wait commands"); split excess waits onto preceding NoOps.
def _split_waits(nc, maxw=1):
    for f in nc.m.functions:
        for bb in f.blocks:
            out = []
            for inst in bb.instructions:
                si = getattr(inst, 'sync_info', None)
                if si is not None and si.on_wait and len(si.on_wait) > maxw:
                    waits = list(si.on_wait)
                    extra, keep = waits[:-maxw], waits[-maxw:]
                    for i in range(0, len(extra), maxw):
                        out.append(mybir.InstNoOp(
                            name=f"{inst.name}-wsplit{i}",
                            sync_info=mybir.SyncInfo(
                                on_wait=extra[i:i + maxw], on_update=[]),
                            bass_nofuse=True,
                            engine=inst.engine,
                        ))
                    inst.sync_info = mybir.SyncInfo(
                        on_wait=keep, on_update=list(si.on_update or []))
                out.append(inst)
            bb.instructions[:] = out


def build_nc(split=True):
    nc = bass.Bass()
    xT = nc.dram_tensor("xT", [C, T], BF16, kind="ExternalInput")
    wq = nc.dram_tensor("wq", [C, 256], BF16, kind="ExternalInput")
    wk = nc.dram_tensor("wk", [C, 256], BF16, kind="ExternalInput")
    wv = nc.dram_tensor("wv", [C, 260], BF16, kind="ExternalInput")
    bqk = nc.dram_tensor("bqk", [P, 4], F32, kind="ExternalInput")  # qA qB kA kB
    scs = nc.dram_tensor("scs", [P, 2 * T], BF16, kind="ExternalInput")  # sin|cos
    wout = nc.dram_tensor("wout", [256, C], BF16, kind="ExternalInput")
    y = nc.dram_tensor("y", [T, C], F32, kind="ExternalOutput")
    masks_d = nc.inline_tensor(_tril_mask(), name="cmasks")

    with tile.TileContext(nc) as tc:
        with ExitStack() as ctx:
            # ---- resident pools ----
            wpool = ctx.enter_context(tc.tile_pool(name="wts", bufs=1))
            qkpool = ctx.enter_context(tc.tile_pool(name="qk", bufs=1))
            vpool = ctx.enter_context(tc.tile_pool(name="v", bufs=1))
            otpool = ctx.enter_context(tc.tile_pool(name="ot", bufs=1))

            bqk_sb = wpool.tile([P, 4], F32, tag="bqk")
            nc.sync.dma_start(bqk_sb[:], bqk[:])
            wq_sb = wpool.tile([P, NCC, 256], BF16, tag="wq")
            nc.sync.dma_start(wq_sb[:], wq.rearrange("(o p) n -> p o n", p=P))
            wk_sb = wpool.tile([P, NCC, 256], BF16, tag="wk")
            wv_sb = wpool.tile([P, NCC, 260], BF16, tag="wv")
            scs_sb = wpool.tile([P, 2 * T], BF16, tag="scs")
            sin4 = scs_sb[:, 0:T]
            cos4 = scs_sb[:, T:2 * T]
            wout_sb = wpool.tile([P, 2, C], BF16, tag="wout")
            masks_f = wpool.tile([P, P], F32, tag="masksf")
            masks_sb = wpool.tile([P, P], BF16, tag="masks")
            ones_f = wpool.tile([P, 64], F32, tag="onesf")
            nc.vector.memset(ones_f[:], 1.0)
            ones_sb = wpool.tile([P, 64], F32R, tag="ones")
            nc.vector.tensor_copy(ones_sb[:], ones_f[:])

            # q/k in rotary-half layout: A = low halves of 4 heads, B = high
            qA = qkpool.tile([P, T], BF16, tag="qA")
            qB = qkpool.tile([P, T], BF16, tag="qB")
            kA = qkpool.tile([P, T], BF16, tag="kA")
            kB = qkpool.tile([P, T], BF16, tag="kB")
            qk_tiles = [qA, qB, kA, kB]
            w_of = {0: wq_sb, 1: wq_sb, 2: wk_sb, 3: wk_sb}
            col_of = {0: 0, 1: 128, 2: 0, 3: 128}

            # V tiles [t-block, 4*65] (65th col per head becomes ones)
            v_tiles = [vpool.tile([P, 260], BF16, tag=f"v{tb}", name=f"v{tb}")
                       for tb in range(NTB)]

            # O^T: heads 0,1 stacked / heads 2,3 stacked (divided, bf16)
            ot_sb = [otpool.tile([P, T], BF16, tag=f"otsb{i}", name=f"otsb{i}")
                     for i in range(2)]

            with ExitStack() as stream:
                xpool = stream.enter_context(tc.tile_pool(name="x", bufs=2))
                rtmp = stream.enter_context(tc.tile_pool(name="rtmp", bufs=4))
                ps_s = stream.enter_context(
                    tc.tile_pool(name="pss", bufs=2, space="PSUM"))
                ps_ot = stream.enter_context(
                    tc.tile_pool(name="psot", bufs=1, space="PSUM"))
                ps_inj = stream.enter_context(
                    tc.tile_pool(name="psinj", bufs=2, space="PSUM"))
                espool = stream.enter_context(tc.tile_pool(name="es", bufs=44))
                dpool = stream.enter_context(tc.tile_pool(name="dv", bufs=3))
                opool = stream.enter_context(tc.tile_pool(name="osb", bufs=2))

                # one-time mask convert fp32 -> bf16
                nc.sync.dma_start(masks_f[:], masks_d[:])
                nc.vector.tensor_copy(masks_sb[:], masks_f[:])

                xt = {}

                def load_xt(tt):
                    for cc in range(NCC):
                        t = xpool.tile([P, QT], BF16, tag=f"x{cc}",
                                       name=f"x{cc}_{tt}")
                        nc.sync.dma_start(
                            t[:], xT[cc * P:(cc + 1) * P,
                                     tt * QT:(tt + 1) * QT])
                        xt[(cc, tt)] = t

                # q/k projection for one (jb, tt): one 8-chunk PSUM
                # generation, bias-add on DVE straight out of PSUM.
                def qkproj(jb, tt):
                    wsb, c0 = w_of[jb], col_of[jb]
                    dst = qk_tiles[jb][:, tt * QT:(tt + 1) * QT]
                    ps = ps_inj.tile([P, QT], F32, tag="inj",
                                     name="qk_ps")
                    for cc in range(NCC):
                        nc.tensor.matmul(
                            ps, wsb[:, cc, c0:c0 + 128], xt[(cc, tt)][:],
                            start=(cc == 0), stop=(cc == NCC - 1))
                    nc.vector.tensor_scalar_add(dst, ps, bqk_sb[:, jb:jb + 1])

                def rope(pair, tt):
                    At, Bt = qk_tiles[2 * pair], qk_tiles[2 * pair + 1]
                    s = slice(tt * QT, (tt + 1) * QT)
                    t1 = rtmp.tile([P, QT], F32, tag="rt", name="rt1")
                    t2 = rtmp.tile([P, QT], F32, tag="rt", name="rt2")
                    t3 = rtmp.tile([P, QT], F32, tag="rt", name="rt3")
                    nc.gpsimd.tensor_mul(t1[:], At[:, s], cos4[:, s])
                    nc.gpsimd.tensor_mul(t2[:], Bt[:, s], sin4[:, s])
                    nc.vector.tensor_mul(t3[:], At[:, s], sin4[:, s])
                    nc.vector.tensor_sub(At[:, s], t1[:], t2[:])
                    nc.gpsimd.tensor_mul(Bt[:, s], Bt[:, s], cos4[:, s])
                    nc.gpsimd.tensor_add(Bt[:, s], Bt[:, s], t3[:])

                def vproj(tb):
                    ps = ps_inj.tile([P, QT], F32, tag="inj",
                                     name="psv")[:, 0:260]
                    for cc in range(NCC):
                        nc.tensor.matmul(
                            ps, xt[(cc, tb // 4)][:, (tb % 4) * P:
                                                  (tb % 4 + 1) * P],
                            wv_sb[:, cc, :],
                            start=(cc == 0), stop=(cc == NCC - 1))
                    nc.scalar.activation(
                        v_tiles[tb][:], ps,
                        mybir.ActivationFunctionType.Copy)
                    ones_cols = v_tiles[tb].rearrange(
                        "p (h e) -> p h e", e=65)[:, :, 64]
                    nc.gpsimd.tensor_scalar_add(ones_cols, ones_cols, 1.0)

                def drain(acc):
                    # copy one (tt, pp) O^T accumulator out of PSUM and
                    # invert the denominator row in place (1-partition DVE
                    # op — no DMA round-trip that could block an engine
                    # queue); frees the psot slot.
                    otf2 = dpool.tile([65, 2, QT], F32, tag="otf",
                                      name="otf")
                    nc.vector.tensor_copy(otf2[:], acc[:])
                    rr = dpool.tile([1, 2, QT], F32R, tag="rr", name="rr")
                    with nc.allow_low_precision(reason="f32r recip for PE"):
                        nc.vector.reciprocal(rr[:], otf2[64:65, :, :])
                    return (otf2, rr)

                def divide_j(tt, pp, pend, j):
                    otf2, rr = pend
                    rb = ps_inj.tile([P, QT], F32, tag="inj",
                                     name="rb")[0:64, :]
                    nc.tensor.matmul(
                        rb, ones_sb[0:1, 0:64],
                        rr[0:1, j, :], start=True, stop=True)
                    dst = ot_sb[pp][64 * j:64 * j + 64,
                                    tt * QT:(tt + 1) * QT]
                    nc.vector.tensor_mul(dst, otf2[0:64, j, :], rb)

                def outproj(tb):
                    o_sb = opool.tile([P, C], F32, tag="osb", name="osb")
                    for nt in range(2):
                        ps = ps_inj.tile([P, QT], F32, tag="inj",
                                          name="pso")
                        for rc in range(2):
                            nc.tensor.matmul(
                                ps, ot_sb[rc][:, tb * P:(tb + 1) * P],
                                wout_sb[:, rc, nt * 512:(nt + 1) * 512],
                                start=(rc == 0), stop=(rc == 1))
                        nc.vector.tensor_copy(
                            o_sb[:, nt * 512:(nt + 1) * 512], ps)
                    nc.gpsimd.dma_start(y[tb * P:(tb + 1) * P, :], o_sb[:])

                # ---- prologue: tile 0's inputs and projections ----
                load_xt(0)
                nc.sync.dma_start(wk_sb[:], wk.rearrange("(o p) n -> p o n", p=P))
                nc.sync.dma_start(scs_sb[:], scs[:])
                nc.sync.dma_start(wv_sb[:], wv.rearrange("(o p) n -> p o n", p=P))
                nc.sync.dma_start(wout_sb[:],
                                  wout.rearrange("(o p) n -> p o n", p=P))
                for jb in range(4):
                    qkproj(jb, 0)
                rope(0, 0)
                rope(1, 0)
                for tb in range(4):
                    vproj(tb)

                # ---- streaming attention with a deferred work queue ----
                # The S^T/exp rounds run on a dedicated PSUM ring and are
                # never blocked by other work.  Everything else — the O^T
                # accumulation (one (tt, pp) phase at a time through the
                # single psot slot), its drain/divide, the next tile's
                # projections, and the out-projection — flows through a FIFO
                # of closures popped a few per round.
                queue = []

                def ot_unit(tt, pp, kblk, es2, nk):
                    off = max(0, (kblk - 4 * tt)) * P

                    def go(acc=None):
                        for j in range(2):
                            h = 2 * pp + j
                            nc.tensor.matmul(
                                acc[:, j, off:],
                                v_tiles[kblk][:, 65 * h:65 * h + 65],
                                es2[:, j, off:],
                                start=(kblk == 0), stop=(kblk == nk - 1))
                    return go

                def pop(n):
                    for _ in range(min(n, len(queue))):
                        queue.pop(0)()

                # phase state: the psot slot cycles through (tt, pp) phases;
                # each phase's accumulator is allocated lazily by its first
                # popped unit (FIFO order sequences it after the previous
                # phase's drain).
                def phase_start(tt, pp, es2, nk, box):
                    def first():
                        box['acc'] = ps_ot.tile([65, 2, QT], F32, tag="psot",
                                                name=f"ps_{tt}_{pp}")
                        ot_unit(tt, pp, 0, es2, nk)(box['acc'])
                    return first

                def phase_end(tt, pp, box):
                    def dr():
                        box['pend'] = drain(box['acc'])
                    return ([dr] +
                            [(lambda j=j: divide_j(tt, pp, box['pend'], j))
                             for j in range(2)])

                es_hist = {}
                for tt in range(TT):
                    nk = 4 * tt + 4
                    box0, box1 = {}, {}
                    if tt + 1 < TT:
                        ntt = tt + 1
                        load_xt(ntt)
                        queue += [lambda jb=jb, t=ntt: qkproj(jb, t)
                                  for jb in range(4)]
                        queue += [lambda t=ntt: rope(0, t),
                                  lambda t=ntt: rope(1, t)]
                        queue += [lambda tb=tb: vproj(tb)
                                  for tb in range(4 * ntt, 4 * ntt + 4)]
                    # everything queued so far (incl. the next tile's
                    # projections) must be emitted before the next tile's
                    # rounds: deps are emission-order-based, so a round
                    # instruction emitted before its producer would race.
                    need = len(queue)
                    popped = 0
                    for kblk in range(nk):
                        off = max(0, (kblk - 4 * tt)) * P
                        ks = slice(kblk * P, (kblk + 1) * P)
                        qs = slice(tt * QT + off, (tt + 1) * QT)
                        # both pps' S^T tiles up front; all four A-half
                        # matmuls back-to-back (4 distinct PE row groups run
                        # concurrently), then the four B-halves.
                        s2s = [ps_s.tile([P, 2, QT], F32, tag="pss",
                                         name="pss") for _ in range(2)]
                        for half, (t_a, t_b) in enumerate(
                                [(kA, qA), (kB, qB)]):
                            for pp in range(2):
                                for j in range(2):
                                    h = 2 * pp + j
                                    hp = slice(32 * h, 32 * h + 32)
                                    nc.tensor.matmul(
                                        s2s[pp][:, j, off:],
                                        t_a[hp, ks], t_b[hp, qs],
                                        start=(half == 0), stop=(half == 1),
                                        tile_position=(32 * h, 0))
                        for pp in range(2):
                            es2 = espool.tile([P, 2, QT], BF16, tag="es",
                                              name="es")
                            nc.scalar.activation(
                                es2[:, :, off:], s2s[pp][:, :, off:],
                                mybir.ActivationFunctionType.Exp, scale=SCALE)
                            if kblk >= 4 * tt:
                                nc.gpsimd.tensor_mul(
                                    es2[:, :, off:off + P],
                                    es2[:, :, off:off + P],
                                    masks_sb[:, None, :].to_broadcast(
                                        (P, 2, P)))
                            es_hist[(tt, kblk, pp)] = es2
                            # queue pp0's O^T for this block as soon as its
                            # es exists; pp1's phase runs after pp0 drains.
                            if pp == 0:
                                if kblk == 0:
                                    queue.append(
                                        phase_start(tt, 0, es2, nk, box0))
                                else:
                                    queue.append(
                                        lambda k=kblk, e=es2, t=tt, n=nk,
                                        b=box0:
                                        ot_unit(t, 0, k, e, n)(b['acc']))
                        # steady drain: clear the must-pop prefix smoothly
                        # across this tile's rounds, at least 2 per round to
                        # keep the PE fed
                        rleft = nk - 1 - kblk
                        deficit = need - popped
                        npop = (-(-deficit // (rleft + 1)) if deficit > 0
                                else 0)
                        npop = min(max(npop, 2), max(6, npop))
                        n0 = len(queue)
                        pop(npop)
                        popped += n0 - len(queue)
                    if popped < need:
                        pop(need - popped)
                    # queue pp0's drain, then the pp1 phase, then outproj
                    queue += phase_end(tt, 0, box0)
                    queue.append(phase_start(tt, 1, es_hist[(tt, 0, 1)],
                                             nk, box1))
                    queue += [lambda k=k, e=es_hist[(tt, k, 1)], t=tt, n=nk,
                              b=box1: ot_unit(t, 1, k, e, n)(b['acc'])
                              for k in range(1, nk)]
                    queue += phase_end(tt, 1, box1)
                    queue += [lambda tb=tb: outproj(tb)
                              for tb in range(4 * tt, 4 * tt + 4)]
                # flush
                pop(len(queue))

    if split:
        _split_waits(nc)
    return nc


def make_in_maps(x, rope_cache, Wqkv, bqkv, Wout, bout):
    """Host-side shard prep. Returns list of 8 in_maps (core = 4*b + g)."""
    x = np.asarray(x, np.float32)
    rope_cache = np.asarray(rope_cache, np.float32)
    Wqkv = np.asarray(Wqkv, np.float32)
    bqkv = np.asarray(bqkv, np.float32)
    Wout = np.asarray(Wout, np.float32)

    # rotary-half permutation within a head: [evens, odds]
    perm = np.concatenate([np.arange(0, D, 2), np.arange(1, D, 2)])
    sin = rope_cache[:, 0::2].T.copy()   # [32, T]
    cos = rope_cache[:, 1::2].T.copy()
    scs = np.concatenate([np.tile(sin, (4, 1)), np.tile(cos, (4, 1))],
                         axis=1).astype(BF)  # [128, 2T]

    xT = [np.ascontiguousarray(x[b].T.astype(BF)) for b in range(B)]

    in_maps = []
    for core in range(N_CORES):
        b, g = divmod(core, G)
        heads = range(HPC * g, HPC * g + HPC)
        # A-block: low halves (even dims) of the 4 heads; B-block: high halves
        qcols, kcols, vcols = [], [], []
        for part in range(2):  # lo, hi
            for h in heads:
                dd = h * D + perm[part * 32:(part + 1) * 32]
                qcols.extend(0 * C + dd)
                kcols.extend(1 * C + dd)
        for h in heads:
            vcols.extend(2 * C + h * D + np.arange(D))
        qcols = np.asarray(qcols)
        kcols = np.asarray(kcols)
        vcols = np.asarray(vcols)
        wq_c = np.ascontiguousarray(Wqkv[:, qcols].astype(BF))
        wk_c = np.ascontiguousarray(Wqkv[:, kcols].astype(BF))
        wv_c = np.zeros((C, 260), np.float32)
        vv = Wqkv[:, vcols]
        for h in range(HPC):
            wv_c[:, 65 * h:65 * h + 64] = vv[:, 64 * h:64 * h + 64]
        bqk_c = np.stack([bqkv[qcols[:128]], bqkv[qcols[128:]],
                          bqkv[kcols[:128]], bqkv[kcols[128:]]], axis=1)
        rows = np.arange(HPC * g * D, (HPC * g + HPC) * D)
        wout_c = np.ascontiguousarray(Wout[rows, :].astype(BF))
        in_maps.append({
            "xT": xT[b], "wq": wq_c, "wk": wk_c,
            "wv": np.ascontiguousarray(wv_c.astype(BF)),
            "bqk": np.ascontiguousarray(bqk_c.astype(np.float32)),
            "scs": scs, "wout": wout_c,
        })
    return in_maps


_NC_CACHE = None


def _get_nc():
    global _NC_CACHE
    if _NC_CACHE is None:
        _NC_CACHE = build_nc()
    return _NC_CACHE


def run(inputs, trace=False):
    nc = _get_nc()
    in_maps = make_in_maps(**inputs)
    res = run_bass_kernel_spmd(nc, in_maps, list(range(N_CORES)), trace=trace)
    bqkv = np.asarray(inputs["bqkv"], np.float32)
    Wout = np.asarray(inputs["Wout"], np.float32)
    bout = np.asarray(inputs["bout"], np.float32)
    # bv commutes through the softmax-weighted average (weights sum to 1):
    # each head's output gains +bv_head, so the final bias is bv @ Wout + bout.
    ybias = bqkv[2 * C:] @ Wout + bout
    out = np.zeros((B, T, C), np.float32)
    for core in range(N_CORES):
        out[core // G] += res.results[core]["y"]
    out += ybias[None, None, :]
    return out, res


def kernel(**inputs):
    out, _ = run(inputs)
    return out


# revision 25
# speedup vs baseline: 1.2625x; 1.2625x over previous
"""Causal self-attention (B=2, T=2048, C=1024, H=16, D=64) on 8 NeuronCores.

Sharding: core = (batch b, head-group g); each of the 8 cores handles one
batch and 4 of the 16 heads (data parallel on B, tensor parallel on heads).
Each core computes q/k/v projections for its heads, rope, causal softmax
attention, and a partial out-projection; the host sums the 4 per-batch
partials and adds bout + bv@Wout (the V-bias commutes through the softmax
because the attention weights sum to 1).

Device dataflow (per core), matmul operands in bf16 (PSUM accumulates fp32;
bf16 halves DMA, enables fast weight loads, and keeps the PE at full clock):
  - host passes x[b].T in bf16 so the contraction dim (C) lands on partitions
  - q,k are produced directly in [dim, t] layout ("A"=low rotary halves of
    4 heads stacked, "B"=high halves); bias-add on DVE, rope on DVE/GpSimd
    with fp32 intermediates
  - S^T = K~^T Q~ per 128k x 512q block, 4 heads packed into the PE array
    via tile_position row groups (contraction=32 each for A/B parts)
  - softmax without max-subtraction (logits are O(1) for this model):
    exp on ScalarE with the 1/8 scale folded in, bf16 output; causal
    masking by multiplying diagonal blocks with constant 0/1 masks
  - O^T = V_aug^T expS^T accumulated over k blocks, where V_aug carries a
    ones column so row 64 of the PSUM accumulator is the softmax
    denominator; divide via DVE reciprocal + K=1 PE broadcast of the recip
    (the DVE multiply reads the broadcast straight from PSUM)
  - partial out-projection [t,c] = (O^T)^T Wout_rows, DMA'd out in fp32
"""
import sys
sys.path.insert(0, '/opt/trn_rl_repo')

import numpy as np
import ml_dtypes
from contextlib import ExitStack

import concourse.bass as bass
import concourse.tile as tile
from concourse import mybir
from concourse.bass_utils import run_bass_kernel_spmd

B, T, C, H, D = 2, 2048, 1024, 16, 64
HPC = 4          # heads per core
G = H // HPC     # head groups (cores per batch)
N_CORES = B * G
SCALE = 1.0 / np.sqrt(D)
P = 128
QT = 512         # q tile width
TT = T // QT     # q tiles
NKB = T // P     # 128-wide k blocks
NTB = T // P     # 128-wide t blocks
NCC = C // P     # 128-deep contraction chunks
F32 = mybir.dt.float32
F32R = mybir.dt.float32r
BF16 = mybir.dt.bfloat16
BF = ml_dtypes.bfloat16


def _tril_mask():
    p = np.arange(P)[:, None]
    f = np.arange(P)[None, :]
    return (p <= f).astype(np.float32)


# walrus in this toolchain can't encode >1 sem wait on one instruction
# ("Too many sync wait commands"); split excess waits onto preceding NoOps.
def _split_waits(nc, maxw=1):
    for f in nc.m.functions:
        for bb in f.blocks:
            out = []
            for inst in bb.instructions:
                si = getattr(inst, 'sync_info', None)
                if si is not None and si.on_wait and len(si.on_wait) > maxw:
                    waits = list(si.on_wait)
                    extra, keep = waits[:-maxw], waits[-maxw:]
                    for i in range(0, len(extra), maxw):
                        out.append(mybir.InstNoOp(
                            name=f"{inst.name}-wsplit{i}",
                            sync_info=mybir.SyncInfo(
                                on_wait=extra[i:i + maxw], on_update=[]),
                            bass_nofuse=True,
                            engine=inst.engine,
                        ))
                    inst.sync_info = mybir.SyncInfo(
                        on_wait=keep, on_update=list(si.on_update or []))
                out.append(inst)
            bb.instructions[:] = out


def build_nc(split=True):
    nc = bass.Bass()
    xT = nc.dram_tensor("xT", [C, T], BF16, kind="ExternalInput")
    wq = nc.dram_tensor("wq", [C, 256], BF16, kind="ExternalInput")
    wk = nc.dram_tensor("wk", [C, 256], BF16, kind="ExternalInput")
    wv = nc.dram_tensor("wv", [C, 260], BF16, kind="ExternalInput")
    bqk = nc.dram_tensor("bqk", [P, 4], F32, kind="ExternalInput")  # qA qB kA kB
    scs = nc.dram_tensor("scs", [P, 2 * T], BF16, kind="ExternalInput")  # sin|cos
    wout = nc.dram_tensor("wout", [256, C], BF16, kind="ExternalInput")
    y = nc.dram_tensor("y", [T, C], F32, kind="ExternalOutput")
    masks_d = nc.inline_tensor(_tril_mask(), name="cmasks")

    with tile.TileContext(nc) as tc:
        with ExitStack() as ctx:
            # ---- resident pools ----
            wpool = ctx.enter_context(tc.tile_pool(name="wts", bufs=1))
            qkpool = ctx.enter_context(tc.tile_pool(name="qk", bufs=1))
            vpool = ctx.enter_context(tc.tile_pool(name="v", bufs=1))
            otpool = ctx.enter_context(tc.tile_pool(name="ot", bufs=1))

            bqk_sb = wpool.tile([P, 4], F32, tag="bqk")
            nc.sync.dma_start(bqk_sb[:], bqk[:])
            wq_sb = wpool.tile([P, NCC, 256], BF16, tag="wq")
            nc.sync.dma_start(wq_sb[:], wq.rearrange("(o p) n -> p o n", p=P))
            wk_sb = wpool.tile([P, NCC, 256], BF16, tag="wk")
            wv_sb = wpool.tile([P, NCC, 260], BF16, tag="wv")
            scs_sb = wpool.tile([P, 2 * T], BF16, tag="scs")
            sin4 = scs_sb[:, 0:T]
            cos4 = scs_sb[:, T:2 * T]
            wout_sb = wpool.tile([P, 2, C], BF16, tag="wout")
            masks_f = wpool.tile([P, P], F32, tag="masksf")
            masks_sb = wpool.tile([P, P], BF16, tag="masks")
            ones_f = wpool.tile([P, 64], F32, tag="onesf")
            nc.vector.memset(ones_f[:], 1.0)
            ones_sb = wpool.tile([P, 64], F32R, tag="ones")
            nc.vector.tensor_copy(ones_sb[:], ones_f[:])

            # q/k in rotary-half layout: A = low halves of 4 heads, B = high
            qA = qkpool.tile([P, T], BF16, tag="qA")
            qB = qkpool.tile([P, T], BF16, tag="qB")
            kA = qkpool.tile([P, T], BF16, tag="kA")
            kB = qkpool.tile([P, T], BF16, tag="kB")
            qk_tiles = [qA, qB, kA, kB]
            w_of = {0: wq_sb, 1: wq_sb, 2: wk_sb, 3: wk_sb}
            col_of = {0: 0, 1: 128, 2: 0, 3: 128}

            # V tiles [t-block, 4*65] (65th col per head becomes ones)
            v_tiles = [vpool.tile([P, 260], BF16, tag=f"v{tb}", name=f"v{tb}")
                       for tb in range(NTB)]

            # O^T: heads 0,1 stacked / heads 2,3 stacked (divided, bf16)
            ot_sb = [otpool.tile([P, T], BF16, tag=f"otsb{i}", name=f"otsb{i}")
                     for i in range(2)]

            with ExitStack() as stream:
                xpool = stream.enter_context(tc.tile_pool(name="x", bufs=2))
                rtmp = stream.enter_context(tc.tile_pool(name="rtmp", bufs=4))
                ps_s = stream.enter_context(
                    tc.tile_pool(name="pss", bufs=2, space="PSUM"))
                ps_ot = stream.enter_context(
                    tc.tile_pool(name="psot", bufs=1, space="PSUM"))
                ps_inj = stream.enter_context(
                    tc.tile_pool(name="psinj", bufs=2, space="PSUM"))
                espool = stream.enter_context(tc.tile_pool(name="es", bufs=44))
                dpool = stream.enter_context(tc.tile_pool(name="dv", bufs=3))
                opool = stream.enter_context(tc.tile_pool(name="osb", bufs=2))

                # one-time mask convert fp32 -> bf16
                nc.sync.dma_start(masks_f[:], masks_d[:])
                nc.vector.tensor_copy(masks_sb[:], masks_f[:])

                xt = {}

                def load_xt(tt):
                    for cc in range(NCC):
                        t = xpool.tile([P, QT], BF16, tag=f"x{cc}",
                                       name=f"x{cc}_{tt}")
                        nc.sync.dma_start(
                            t[:], xT[cc * P:(cc + 1) * P,
                                     tt * QT:(tt + 1) * QT])
                        xt[(cc, tt)] = t

                # q/k projection for one (jb, tt): one 8-chunk PSUM
                # generation, bias-add on DVE straight out of PSUM.
                def qkproj(jb, tt):
                    wsb, c0 = w_of[jb], col_of[jb]
                    dst = qk_tiles[jb][:, tt * QT:(tt + 1) * QT]
                    ps = ps_inj.tile([P, QT], F32, tag="inj",
                                     name="qk_ps")
                    for cc in range(NCC):
                        nc.tensor.matmul(
                            ps, wsb[:, cc, c0:c0 + 128], xt[(cc, tt)][:],
                            start=(cc == 0), stop=(cc == NCC - 1))
                    nc.vector.tensor_scalar_add(dst, ps, bqk_sb[:, jb:jb + 1])

                def rope(pair, tt):
                    At, Bt = qk_tiles[2 * pair], qk_tiles[2 * pair + 1]
                    s = slice(tt * QT, (tt + 1) * QT)
                    t1 = rtmp.tile([P, QT], F32, tag="rt", name="rt1")
                    t2 = rtmp.tile([P, QT], F32, tag="rt", name="rt2")
                    t3 = rtmp.tile([P, QT], F32, tag="rt", name="rt3")
                    nc.gpsimd.tensor_mul(t1[:], At[:, s], cos4[:, s])
                    nc.gpsimd.tensor_mul(t2[:], Bt[:, s], sin4[:, s])
                    nc.vector.tensor_mul(t3[:], At[:, s], sin4[:, s])
                    nc.vector.tensor_sub(At[:, s], t1[:], t2[:])
                    nc.gpsimd.tensor_mul(Bt[:, s], Bt[:, s], cos4[:, s])
                    nc.gpsimd.tensor_add(Bt[:, s], Bt[:, s], t3[:])

                def vproj(tb):
                    ps = ps_inj.tile([P, QT], F32, tag="inj",
                                     name="psv")[:, 0:260]
                    for cc in range(NCC):
                        nc.tensor.matmul(
                            ps, xt[(cc, tb // 4)][:, (tb % 4) * P:
                                                  (tb % 4 + 1) * P],
                            wv_sb[:, cc, :],
                            start=(cc == 0), stop=(cc == NCC - 1))
                    nc.scalar.activation(
                        v_tiles[tb][:], ps,
                        mybir.ActivationFunctionType.Copy)
                    ones_cols = v_tiles[tb].rearrange(
                        "p (h e) -> p h e", e=65)[:, :, 64]
                    nc.gpsimd.tensor_scalar_add(ones_cols, ones_cols, 1.0)

                def drain(acc):
                    # copy one (tt, pp) O^T accumulator out of PSUM and set
                    # up the reciprocal row via a partition-spread DMA round
                    # trip (the consumers pop several rounds later, hiding
                    # the DMA latency); frees the psot slot.
                    otf2 = dpool.tile([65, 2, QT], F32, tag="otf",
                                      name="otf")
                    nc.vector.tensor_copy(otf2[:], acc[:])
                    dn = dpool.tile([P, 8], F32, tag="dn", name="dn")
                    nc.sync.dma_start(
                        dn[:], otf2[64:65, :, :].rearrange(
                            "a b c -> a (b c)"))
                    nc.vector.reciprocal(dn[:], dn[:])
                    rr = dpool.tile([1, 2, QT], F32R, tag="rr", name="rr")
                    nc.sync.dma_start(
                        rr[0:1, :, :].rearrange("a b c -> a (b c)"),
                        dn[:].bitcast(F32R))
                    return (otf2, rr)

                def divide_j(tt, pp, pend, j):
                    otf2, rr = pend
                    rb = ps_inj.tile([P, QT], F32, tag="inj",
                                     name="rb")[0:64, :]
                    nc.tensor.matmul(
                        rb, ones_sb[0:1, 0:64],
                        rr[0:1, j, :], start=True, stop=True)
                    dst = ot_sb[pp][64 * j:64 * j + 64,
                                    tt * QT:(tt + 1) * QT]
                    nc.vector.tensor_mul(dst, otf2[0:64, j, :], rb)

                def outproj(tb):
                    o_sb = opool.tile([P, C], F32, tag="osb", name="osb")
                    for nt in range(2):
                        ps = ps_inj.tile([P, QT], F32, tag="inj",
                                          name="pso")
                        for rc in range(2):
                            nc.tensor.matmul(
                                ps, ot_sb[rc][:, tb * P:(tb + 1) * P],
                                wout_sb[:, rc, nt * 512:(nt + 1) * 512],
                                start=(rc == 0), stop=(rc == 1))
                        nc.vector.tensor_copy(
                            o_sb[:, nt * 512:(nt + 1) * 512], ps)
                    nc.gpsimd.dma_start(y[tb * P:(tb + 1) * P, :], o_sb[:])

                # ---- prologue: tile 0's inputs and projections ----
                load_xt(0)
                nc.sync.dma_start(wk_sb[:], wk.rearrange("(o p) n -> p o n", p=P))
                nc.sync.dma_start(scs_sb[:], scs[:])
                nc.sync.dma_start(wv_sb[:], wv.rearrange("(o p) n -> p o n", p=P))
                nc.sync.dma_start(wout_sb[:],
                                  wout.rearrange("(o p) n -> p o n", p=P))
                for jb in range(4):
                    qkproj(jb, 0)
                rope(0, 0)
                rope(1, 0)
                for tb in range(4):
                    vproj(tb)

                # ---- streaming attention with a deferred work queue ----
                # The S^T/exp rounds run on a dedicated PSUM ring and are
                # never blocked by other work.  Everything else — the O^T
                # accumulation (one (tt, pp) phase at a time through the
                # single psot slot), its drain/divide, the next tile's
                # projections, and the out-projection — flows through a FIFO
                # of closures popped a few per round.
                queue = []

                def ot_unit(tt, pp, kblk, es2, nk):
                    off = max(0, (kblk - 4 * tt)) * P

                    def go(acc=None):
                        for j in range(2):
                            h = 2 * pp + j
                            nc.tensor.matmul(
                                acc[:, j, off:],
                                v_tiles[kblk][:, 65 * h:65 * h + 65],
                                es2[:, j, off:],
                                start=(kblk == 0), stop=(kblk == nk - 1))
                    return go

                def pop(n):
                    for _ in range(min(n, len(queue))):
                        queue.pop(0)()

                # phase state: the psot slot cycles through (tt, pp) phases;
                # each phase's accumulator is allocated lazily by its first
                # popped unit (FIFO order sequences it after the previous
                # phase's drain).
                def phase_start(tt, pp, es2, nk, box):
                    def first():
                        box['acc'] = ps_ot.tile([65, 2, QT], F32, tag="psot",
                                                name=f"ps_{tt}_{pp}")
                        ot_unit(tt, pp, 0, es2, nk)(box['acc'])
                    return first

                def phase_end(tt, pp, box):
                    def dr():
                        box['pend'] = drain(box['acc'])
                    return ([dr] +
                            [(lambda j=j: divide_j(tt, pp, box['pend'], j))
                             for j in range(2)])

                es_hist = {}
                deferred = []
                for tt in range(TT):
                    nk = 4 * tt + 4
                    box0, box1 = {}, {}
                    if tt + 1 < TT:
                        ntt = tt + 1
                        load_xt(ntt)
                        queue += [lambda jb=jb, t=ntt: qkproj(jb, t)
                                  for jb in range(4)]
                        queue += [lambda t=ntt: rope(0, t),
                                  lambda t=ntt: rope(1, t)]
                        queue += [lambda tb=tb: vproj(tb)
                                  for tb in range(4 * ntt, 4 * ntt + 4)]
                    # splice the previous tile's deferred divide/outproj
                    # after the prep so they pop with some spacing
                    queue += deferred
                    deferred = []
                    # everything queued so far (incl. the next tile's
                    # projections) must be emitted before the next tile's
                    # rounds: deps are emission-order-based, so a round
                    # instruction emitted before its producer would race.
                    need = len(queue)
                    popped = 0
                    for kblk in range(nk):
                        off = max(0, (kblk - 4 * tt)) * P
                        ks = slice(kblk * P, (kblk + 1) * P)
                        qs = slice(tt * QT + off, (tt + 1) * QT)
                        # both pps' S^T tiles up front; all four A-half
                        # matmuls back-to-back (4 distinct PE row groups run
                        # concurrently), then the four B-halves.
                        s2s = [ps_s.tile([P, 2, QT], F32, tag="pss",
                                         name="pss") for _ in range(2)]
                        for half, (t_a, t_b) in enumerate(
                                [(kA, qA), (kB, qB)]):
                            for pp in range(2):
                                for j in range(2):
                                    h = 2 * pp + j
                                    hp = slice(32 * h, 32 * h + 32)
                                    nc.tensor.matmul(
                                        s2s[pp][:, j, off:],
                                        t_a[hp, ks], t_b[hp, qs],
                                        start=(half == 0), stop=(half == 1),
                                        tile_position=(32 * h, 0))
                        for pp in range(2):
                            es2 = espool.tile([P, 2, QT], BF16, tag="es",
                                              name="es")
                            nc.scalar.activation(
                                es2[:, :, off:], s2s[pp][:, :, off:],
                                mybir.ActivationFunctionType.Exp, scale=SCALE)
                            if kblk >= 4 * tt:
                                nc.gpsimd.tensor_mul(
                                    es2[:, :, off:off + P],
                                    es2[:, :, off:off + P],
                                    masks_sb[:, None, :].to_broadcast(
                                        (P, 2, P)))
                            es_hist[(tt, kblk, pp)] = es2
                            # queue pp0's O^T for this block as soon as its
                            # es exists; pp1's phase runs after pp0 drains.
                            if pp == 0:
                                if kblk == 0:
                                    queue.append(
                                        phase_start(tt, 0, es2, nk, box0))
                                else:
                                    queue.append(
                                        lambda k=kblk, e=es2, t=tt, n=nk,
                                        b=box0:
                                        ot_unit(t, 0, k, e, n)(b['acc']))
                        # steady drain: clear the must-pop prefix smoothly
                        # across this tile's rounds, at least 2 per round to
                        # keep the PE fed
                        rleft = nk - 1 - kblk
                        deficit = need - popped
                        npop = (-(-deficit // (rleft + 1)) if deficit > 0
                                else 0)
                        npop = min(max(npop, 2), max(6, npop))
                        n0 = len(queue)
                        pop(npop)
                        popped += n0 - len(queue)
                    if popped < need:
                        pop(need - popped)
                    # pp0's drain, then the whole pp1 phase, THEN pp0's
                    # divides (spaced nk units after their drain so the
                    # denominator DMA round-trip is hidden), then pp1's
                    # drain; pp1's divides and the outproj are deferred to
                    # the next tile's queue (spaced behind its projections).
                    dr0, dj0a, dj0b = phase_end(tt, 0, box0)
                    queue.append(dr0)
                    queue.append(phase_start(tt, 1, es_hist[(tt, 0, 1)],
                                             nk, box1))
                    queue += [lambda k=k, e=es_hist[(tt, k, 1)], t=tt, n=nk,
                              b=box1: ot_unit(t, 1, k, e, n)(b['acc'])
                              for k in range(1, nk)]
                    dr1, dj1a, dj1b = phase_end(tt, 1, box1)
                    queue += [dj0a, dj0b, dr1]
                    deferred = [dj1a, dj1b] + [
                        lambda tb=tb: outproj(tb)
                        for tb in range(4 * tt, 4 * tt + 4)]
                # flush (the last tile's deferred work included)
                queue += deferred
                pop(len(queue))

    if split:
        _split_waits(nc)
    return nc


def make_in_maps(x, rope_cache, Wqkv, bqkv, Wout, bout):
    """Host-side shard prep. Returns list of 8 in_maps (core = 4*b + g)."""
    x = np.asarray(x, np.float32)
    rope_cache = np.asarray(rope_cache, np.float32)
    Wqkv = np.asarray(Wqkv, np.float32)
    bqkv = np.asarray(bqkv, np.float32)
    Wout = np.asarray(Wout, np.float32)

    # rotary-half permutation within a head: [evens, odds]
    perm = np.concatenate([np.arange(0, D, 2), np.arange(1, D, 2)])
    sin = rope_cache[:, 0::2].T.copy()   # [32, T]
    cos = rope_cache[:, 1::2].T.copy()
    scs = np.concatenate([np.tile(sin, (4, 1)), np.tile(cos, (4, 1))],
                         axis=1).astype(BF)  # [128, 2T]

    xT = [np.ascontiguousarray(x[b].T.astype(BF)) for b in range(B)]

    in_maps = []
    for core in range(N_CORES):
        b, g = divmod(core, G)
        heads = range(HPC * g, HPC * g + HPC)
        # A-block: low halves (even dims) of the 4 heads; B-block: high halves
        qcols, kcols, vcols = [], [], []
        for part in range(2):  # lo, hi
            for h in heads:
                dd = h * D + perm[part * 32:(part + 1) * 32]
                qcols.extend(0 * C + dd)
                kcols.extend(1 * C + dd)
        for h in heads:
            vcols.extend(2 * C + h * D + np.arange(D))
        qcols = np.asarray(qcols)
        kcols = np.asarray(kcols)
        vcols = np.asarray(vcols)
        wq_c = np.ascontiguousarray(Wqkv[:, qcols].astype(BF))
        wk_c = np.ascontiguousarray(Wqkv[:, kcols].astype(BF))
        wv_c = np.zeros((C, 260), np.float32)
        vv = Wqkv[:, vcols]
        for h in range(HPC):
            wv_c[:, 65 * h:65 * h + 64] = vv[:, 64 * h:64 * h + 64]
        bqk_c = np.stack([bqkv[qcols[:128]], bqkv[qcols[128:]],
                          bqkv[kcols[:128]], bqkv[kcols[128:]]], axis=1)
        rows = np.arange(HPC * g * D, (HPC * g + HPC) * D)
        wout_c = np.ascontiguousarray(Wout[rows, :].astype(BF))
        in_maps.append({
            "xT": xT[b], "wq": wq_c, "wk": wk_c,
            "wv": np.ascontiguousarray(wv_c.astype(BF)),
            "bqk": np.ascontiguousarray(bqk_c.astype(np.float32)),
            "scs": scs, "wout": wout_c,
        })
    return in_maps


_NC_CACHE = None


def _get_nc():
    global _NC_CACHE
    if _NC_CACHE is None:
        _NC_CACHE = build_nc()
    return _NC_CACHE


def run(inputs, trace=False):
    nc = _get_nc()
    in_maps = make_in_maps(**inputs)
    res = run_bass_kernel_spmd(nc, in_maps, list(range(N_CORES)), trace=trace)
    bqkv = np.asarray(inputs["bqkv"], np.float32)
    Wout = np.asarray(inputs["Wout"], np.float32)
    bout = np.asarray(inputs["bout"], np.float32)
    # bv commutes through the softmax-weighted average (weights sum to 1):
    # each head's output gains +bv_head, so the final bias is bv @ Wout + bout.
    ybias = bqkv[2 * C:] @ Wout + bout
    out = np.zeros((B, T, C), np.float32)
    for core in range(N_CORES):
        out[core // G] += res.results[core]["y"]
    out += ybias[None, None, :]
    return out, res


def kernel(**inputs):
    out, _ = run(inputs)
    return out


# revision 28
# speedup vs baseline: 1.3149x; 1.0416x over previous
"""Causal self-attention (B=2, T=2048, C=1024, H=16, D=64) on 8 NeuronCores.

Sharding: core = (batch b, head-group g); each of the 8 cores handles one
batch and 4 of the 16 heads (data parallel on B, tensor parallel on heads).
Each core computes q/k/v projections for its heads, rope, causal softmax
attention, and a partial out-projection; the host sums the 4 per-batch
partials and adds bout + bv@Wout (the V-bias commutes through the softmax
because the attention weights sum to 1).

Device dataflow (per core), matmul operands in bf16 (PSUM accumulates fp32;
bf16 halves DMA, enables fast weight loads, and keeps the PE at full clock):
  - host passes x[b].T in bf16 so the contraction dim (C) lands on partitions
  - q,k are produced directly in [dim, t] layout ("A"=low rotary halves of
    4 heads stacked, "B"=high halves); bias-add on DVE, rope on DVE/GpSimd
    with fp32 intermediates
  - S^T = K~^T Q~ per 128k x 512q block, 4 heads packed into the PE array
    via tile_position row groups (contraction=32 each for A/B parts)
  - softmax without max-subtraction (logits are O(1) for this model):
    exp on ScalarE with the 1/8 scale folded in, bf16 output; causal
    masking by multiplying diagonal blocks with constant 0/1 masks
  - O^T = V_aug^T expS^T accumulated over k blocks, where V_aug carries a
    ones column so row 64 of the PSUM accumulator is the softmax
    denominator; divide via DVE reciprocal + K=1 PE broadcast of the recip
    (the DVE multiply reads the broadcast straight from PSUM)
  - partial out-projection [t,c] = (O^T)^T Wout_rows, DMA'd out in fp32
"""
import sys
sys.path.insert(0, '/opt/trn_rl_repo')

import numpy as np
import ml_dtypes
from contextlib import ExitStack

import concourse.bass as bass
import concourse.tile as tile
from concourse import mybir
from concourse.bass_utils import run_bass_kernel_spmd

B, T, C, H, D = 2, 2048, 1024, 16, 64
HPC = 4          # heads per core
G = H // HPC     # head groups (cores per batch)
N_CORES = B * G
SCALE = 1.0 / np.sqrt(D)
P = 128
QT = 512         # q tile width
TT = T // QT     # q tiles
NKB = T // P     # 128-wide k blocks
NTB = T // P     # 128-wide t blocks
NCC = C // P     # 128-deep contraction chunks
F32 = mybir.dt.float32
F32R = mybir.dt.float32r
BF16 = mybir.dt.bfloat16
BF = ml_dtypes.bfloat16


def _tril_mask():
    p = np.arange(P)[:, None]
    f = np.arange(P)[None, :]
    return (p <= f).astype(np.float32)


# walrus in this toolchain can't encode >1 sem wait on one instruction
# ("Too many sync wait commands"); split excess waits onto preceding NoOps.
def _split_waits(nc, maxw=1):
    for f in nc.m.functions:
        for bb in f.blocks:
            out = []
            for inst in bb.instructions:
                si = getattr(inst, 'sync_info', None)
                if si is not None and si.on_wait and len(si.on_wait) > maxw:
                    waits = list(si.on_wait)
                    extra, keep = waits[:-maxw], waits[-maxw:]
                    for i in range(0, len(extra), maxw):
                        out.append(mybir.InstNoOp(
                            name=f"{inst.name}-wsplit{i}",
                            sync_info=mybir.SyncInfo(
                                on_wait=extra[i:i + maxw], on_update=[]),
                            bass_nofuse=True,
                            engine=inst.engine,
                        ))
                    inst.sync_info = mybir.SyncInfo(
                        on_wait=keep, on_update=list(si.on_update or []))
                out.append(inst)
            bb.instructions[:] = out


def build_nc(split=True):
    nc = bass.Bass()
    xT = nc.dram_tensor("xT", [C, T], BF16, kind="ExternalInput")
    wq = nc.dram_tensor("wq", [C, 256], BF16, kind="ExternalInput")
    wk = nc.dram_tensor("wk", [C, 256], BF16, kind="ExternalInput")
    wv = nc.dram_tensor("wv", [C, 260], BF16, kind="ExternalInput")
    bqk = nc.dram_tensor("bqk", [P, 4], F32, kind="ExternalInput")  # qA qB kA kB
    scs = nc.dram_tensor("scs", [P, 2 * T], BF16, kind="ExternalInput")  # sin|cos
    wout = nc.dram_tensor("wout", [256, C], BF16, kind="ExternalInput")
    y = nc.dram_tensor("y", [T, C], F32, kind="ExternalOutput")
    masks_d = nc.inline_tensor(_tril_mask(), name="cmasks")

    with tile.TileContext(nc) as tc:
        with ExitStack() as ctx:
            # ---- resident pools ----
            wpool = ctx.enter_context(tc.tile_pool(name="wts", bufs=1))
            qkpool = ctx.enter_context(tc.tile_pool(name="qk", bufs=1))
            vpool = ctx.enter_context(tc.tile_pool(name="v", bufs=1))
            otpool = ctx.enter_context(tc.tile_pool(name="ot", bufs=1))

            bqk_sb = wpool.tile([P, 4], F32, tag="bqk")
            nc.sync.dma_start(bqk_sb[:], bqk[:])
            wq_sb = wpool.tile([P, NCC, 256], BF16, tag="wq")
            nc.sync.dma_start(wq_sb[:], wq.rearrange("(o p) n -> p o n", p=P))
            wk_sb = wpool.tile([P, NCC, 256], BF16, tag="wk")
            wv_sb = wpool.tile([P, NCC, 260], BF16, tag="wv")
            scs_sb = wpool.tile([P, 2 * T], BF16, tag="scs")
            sin4 = scs_sb[:, 0:T]
            cos4 = scs_sb[:, T:2 * T]
            wout_sb = wpool.tile([P, 2, C], BF16, tag="wout")
            masks_f = wpool.tile([P, P], F32, tag="masksf")
            masks_sb = wpool.tile([P, P], BF16, tag="masks")
            ones_f = wpool.tile([P, 64], F32, tag="onesf")
            nc.vector.memset(ones_f[:], 1.0)
            ones_sb = wpool.tile([P, 64], F32R, tag="ones")
            nc.vector.tensor_copy(ones_sb[:], ones_f[:])

            # q/k in rotary-half layout: A = low halves of 4 heads, B = high
            qA = qkpool.tile([P, T], BF16, tag="qA")
            qB = qkpool.tile([P, T], BF16, tag="qB")
            kA = qkpool.tile([P, T], BF16, tag="kA")
            kB = qkpool.tile([P, T], BF16, tag="kB")
            qk_tiles = [qA, qB, kA, kB]
            w_of = {0: wq_sb, 1: wq_sb, 2: wk_sb, 3: wk_sb}
            col_of = {0: 0, 1: 128, 2: 0, 3: 128}

            # V tiles [t-block, 4*65] (65th col per head becomes ones)
            v_tiles = [vpool.tile([P, 260], BF16, tag=f"v{tb}", name=f"v{tb}")
                       for tb in range(NTB)]

            # O^T: heads 0,1 stacked / heads 2,3 stacked (divided, bf16)
            ot_sb = [otpool.tile([P, T], BF16, tag=f"otsb{i}", name=f"otsb{i}")
                     for i in range(2)]

            with ExitStack() as stream:
                xpool = stream.enter_context(tc.tile_pool(name="x", bufs=2))
                rtmp = stream.enter_context(tc.tile_pool(name="rtmp", bufs=4))
                ps_s = stream.enter_context(
                    tc.tile_pool(name="pss", bufs=2, space="PSUM"))
                ps_ot = stream.enter_context(
                    tc.tile_pool(name="psot", bufs=1, space="PSUM"))
                ps_inj = stream.enter_context(
                    tc.tile_pool(name="psinj", bufs=2, space="PSUM"))
                espool = stream.enter_context(tc.tile_pool(name="es", bufs=44))
                dpool = stream.enter_context(tc.tile_pool(name="dv", bufs=3))
                opool = stream.enter_context(tc.tile_pool(name="osb", bufs=2))

                # one-time mask convert fp32 -> bf16
                nc.sync.dma_start(masks_f[:], masks_d[:])
                nc.vector.tensor_copy(masks_sb[:], masks_f[:])

                xt = {}

                def load_xt(tt):
                    for cc in range(NCC):
                        t = xpool.tile([P, QT], BF16, tag=f"x{cc}",
                                       name=f"x{cc}_{tt}")
                        nc.sync.dma_start(
                            t[:], xT[cc * P:(cc + 1) * P,
                                     tt * QT:(tt + 1) * QT])
                        xt[(cc, tt)] = t

                # q/k projection for one (jb, tt): one 8-chunk PSUM
                # generation, bias-add on DVE straight out of PSUM.
                def qkproj(jb, tt):
                    wsb, c0 = w_of[jb], col_of[jb]
                    dst = qk_tiles[jb][:, tt * QT:(tt + 1) * QT]
                    ps = ps_inj.tile([P, QT], F32, tag="inj",
                                     name="qk_ps")
                    for cc in range(NCC):
                        nc.tensor.matmul(
                            ps, wsb[:, cc, c0:c0 + 128], xt[(cc, tt)][:],
                            start=(cc == 0), stop=(cc == NCC - 1))
                    nc.vector.tensor_scalar_add(dst, ps, bqk_sb[:, jb:jb + 1])

                def rope(pair, tt):
                    At, Bt = qk_tiles[2 * pair], qk_tiles[2 * pair + 1]
                    s = slice(tt * QT, (tt + 1) * QT)
                    t1 = rtmp.tile([P, QT], F32, tag="rt", name="rt1")
                    t2 = rtmp.tile([P, QT], F32, tag="rt", name="rt2")
                    t3 = rtmp.tile([P, QT], F32, tag="rt", name="rt3")
                    nc.gpsimd.tensor_mul(t1[:], At[:, s], cos4[:, s])
                    nc.gpsimd.tensor_mul(t2[:], Bt[:, s], sin4[:, s])
                    nc.vector.tensor_mul(t3[:], At[:, s], sin4[:, s])
                    nc.vector.tensor_sub(At[:, s], t1[:], t2[:])
                    nc.gpsimd.tensor_mul(Bt[:, s], Bt[:, s], cos4[:, s])
                    nc.gpsimd.tensor_add(Bt[:, s], Bt[:, s], t3[:])

                def vproj(tb):
                    ps = ps_inj.tile([P, QT], F32, tag="inj",
                                     name="psv")[:, 0:260]
                    for cc in range(NCC):
                        nc.tensor.matmul(
                            ps, xt[(cc, tb // 4)][:, (tb % 4) * P:
                                                  (tb % 4 + 1) * P],
                            wv_sb[:, cc, :],
                            start=(cc == 0), stop=(cc == NCC - 1))
                    nc.scalar.activation(
                        v_tiles[tb][:], ps,
                        mybir.ActivationFunctionType.Copy)
                    ones_cols = v_tiles[tb].rearrange(
                        "p (h e) -> p h e", e=65)[:, :, 64]
                    nc.gpsimd.tensor_scalar_add(ones_cols, ones_cols, 1.0)

                def drain(acc):
                    # copy one (tt, pp) O^T accumulator out of PSUM and set
                    # up the reciprocal row via a partition-spread DMA round
                    # trip (the consumers pop several rounds later, hiding
                    # the DMA latency); frees the psot slot.
                    otf2 = dpool.tile([65, 2, QT], F32, tag="otf",
                                      name="otf")
                    nc.vector.tensor_copy(otf2[:], acc[:])
                    dn = dpool.tile([P, 8], F32, tag="dn", name="dn")
                    nc.sync.dma_start(
                        dn[:], otf2[64:65, :, :].rearrange(
                            "a b c -> a (b c)"))
                    nc.vector.reciprocal(dn[:], dn[:])
                    rr = dpool.tile([1, 2, QT], F32R, tag="rr", name="rr")
                    nc.sync.dma_start(
                        rr[0:1, :, :].rearrange("a b c -> a (b c)"),
                        dn[:].bitcast(F32R))
                    return (otf2, rr)

                def divide_j(tt, pp, pend, j):
                    otf2, rr = pend
                    rb = ps_inj.tile([P, QT], F32, tag="inj",
                                     name="rb")[0:64, :]
                    nc.tensor.matmul(
                        rb, ones_sb[0:1, 0:64],
                        rr[0:1, j, :], start=True, stop=True)
                    dst = ot_sb[pp][64 * j:64 * j + 64,
                                    tt * QT:(tt + 1) * QT]
                    nc.vector.tensor_mul(dst, otf2[0:64, j, :], rb)

                def outproj(tb):
                    o_sb = opool.tile([P, C], F32, tag="osb", name="osb")
                    for nt in range(2):
                        ps = ps_inj.tile([P, QT], F32, tag="inj",
                                          name="pso")
                        for rc in range(2):
                            nc.tensor.matmul(
                                ps, ot_sb[rc][:, tb * P:(tb + 1) * P],
                                wout_sb[:, rc, nt * 512:(nt + 1) * 512],
                                start=(rc == 0), stop=(rc == 1))
                        nc.vector.tensor_copy(
                            o_sb[:, nt * 512:(nt + 1) * 512], ps)
                    nc.gpsimd.dma_start(y[tb * P:(tb + 1) * P, :], o_sb[:])

                # ---- prologue: tile 0's inputs and projections ----
                t0 = 0
                load_xt(t0)
                nc.gpsimd.dma_start(wk_sb[:],
                                    wk.rearrange("(o p) n -> p o n", p=P))
                nc.scalar.dma_start(scs_sb[:], scs[:])
                nc.gpsimd.dma_start(wv_sb[:],
                                    wv.rearrange("(o p) n -> p o n", p=P))
                nc.scalar.dma_start(wout_sb[:],
                                    wout.rearrange("(o p) n -> p o n", p=P))
                for jb in range(4):
                    qkproj(jb, t0)
                rope(0, t0)
                rope(1, t0)
                for tb in range(4 * t0, 4 * t0 + 4):
                    vproj(tb)

                # ---- streaming attention with a deferred work queue ----
                # The S^T/exp rounds run on a dedicated PSUM ring and are
                # never blocked by other work.  Everything else — the O^T
                # accumulation (one (tt, pp) phase at a time through the
                # single psot slot), its drain/divide, the next tile's
                # projections, and the out-projection — flows through a FIFO
                # of closures popped a few per round.
                queue = []

                def ot_unit(tt, pp, kblk, es2, nk):
                    off = max(0, (kblk - 4 * tt)) * P

                    def go(acc=None):
                        for j in range(2):
                            h = 2 * pp + j
                            nc.tensor.matmul(
                                acc[:, j, off:],
                                v_tiles[kblk][:, 65 * h:65 * h + 65],
                                es2[:, j, off:],
                                start=(kblk == 0), stop=(kblk == nk - 1))
                    return go

                def pop(n):
                    for _ in range(min(n, len(queue))):
                        queue.pop(0)()

                # phase state: the psot slot cycles through (tt, pp) phases;
                # each phase's accumulator is allocated lazily by its first
                # popped unit (FIFO order sequences it after the previous
                # phase's drain).
                def phase_start(tt, pp, es2, nk, box):
                    def first():
                        box['acc'] = ps_ot.tile([65, 2, QT], F32, tag="psot",
                                                name=f"ps_{tt}_{pp}")
                        ot_unit(tt, pp, 0, es2, nk)(box['acc'])
                    return first

                def phase_end(tt, pp, box):
                    def dr():
                        box['pend'] = drain(box['acc'])
                    return ([dr] +
                            [(lambda j=j: divide_j(tt, pp, box['pend'], j))
                             for j in range(2)])

                es_hist = {}
                deferred = []
                order = list(range(TT))
                rounds_after = [sum(4 * o + 4 for o in order[i:])
                                for i in range(len(order))]
                for oi, tt in enumerate(order):
                    nk = 4 * tt + 4
                    box0, box1 = {}, {}
                    if oi + 1 < len(order):
                        ntt = order[oi + 1]
                        load_xt(ntt)
                        # prep for the NEXT tile goes to the queue FRONT —
                        # it's independent of the backlog and must be fully
                        # emitted before that tile's rounds (deps are
                        # emission-order-based)
                        prep = [lambda jb=jb, t=ntt: qkproj(jb, t)
                                for jb in range(4)]
                        prep += [lambda t=ntt: rope(0, t),
                                 lambda t=ntt: rope(1, t)]
                        prep += [lambda tb=tb: vproj(tb)
                                 for tb in range(4 * ntt, 4 * ntt + 4)]
                        queue[0:0] = prep
                    queue += deferred
                    deferred = []
                    rounds_rem_all = rounds_after[oi]
                    for kblk in range(nk):
                        off = max(0, (kblk - 4 * tt)) * P
                        ks = slice(kblk * P, (kblk + 1) * P)
                        qs = slice(tt * QT + off, (tt + 1) * QT)
                        # both pps' S^T tiles up front; all four A-half
                        # matmuls back-to-back (4 distinct PE row groups run
                        # concurrently), then the four B-halves.
                        s2s = [ps_s.tile([P, 2, QT], F32, tag="pss",
                                         name="pss") for _ in range(2)]
                        for half, (t_a, t_b) in enumerate(
                                [(kA, qA), (kB, qB)]):
                            for pp in range(2):
                                for j in range(2):
                                    h = 2 * pp + j
                                    hp = slice(32 * h, 32 * h + 32)
                                    nc.tensor.matmul(
                                        s2s[pp][:, j, off:],
                                        t_a[hp, ks], t_b[hp, qs],
                                        start=(half == 0), stop=(half == 1),
                                        tile_position=(32 * h, 0))
                        for pp in range(2):
                            es2 = espool.tile([P, 2, QT], BF16, tag="es",
                                              name="es")
                            nc.scalar.activation(
                                es2[:, :, off:], s2s[pp][:, :, off:],
                                mybir.ActivationFunctionType.Exp, scale=SCALE)
                            if kblk >= 4 * tt:
                                nc.gpsimd.tensor_mul(
                                    es2[:, :, off:off + P],
                                    es2[:, :, off:off + P],
                                    masks_sb[:, None, :].to_broadcast(
                                        (P, 2, P)))
                            es_hist[(tt, kblk, pp)] = es2
                            # queue pp0's O^T for this block as soon as its
                            # es exists; pp1's phase runs after pp0 drains.
                            if pp == 0:
                                if kblk == 0:
                                    queue.append(
                                        phase_start(tt, 0, es2, nk, box0))
                                else:
                                    queue.append(
                                        lambda k=kblk, e=es2, t=tt, n=nk,
                                        b=box0:
                                        ot_unit(t, 0, k, e, n)(b['acc']))
                        # steady drain: pace the backlog to finish by the
                        # final round overall; at least 2 per round
                        rem = rounds_rem_all - kblk - 1
                        npop = (-(-len(queue) // rem) if rem > 0
                                else len(queue))
                        pop(max(npop, 2))
                    # pp0's drain, then the whole pp1 phase, THEN pp0's
                    # divides (spaced nk units after their drain so the
                    # denominator DMA round-trip is hidden), then pp1's
                    # drain; pp1's divides and the outproj are deferred to
                    # the next tile's queue (spaced behind its projections).
                    dr0, dj0a, dj0b = phase_end(tt, 0, box0)
                    queue.append(dr0)
                    queue.append(phase_start(tt, 1, es_hist[(tt, 0, 1)],
                                             nk, box1))
                    queue += [lambda k=k, e=es_hist[(tt, k, 1)], t=tt, n=nk,
                              b=box1: ot_unit(t, 1, k, e, n)(b['acc'])
                              for k in range(1, nk)]
                    dr1, dj1a, dj1b = phase_end(tt, 1, box1)
                    queue += [dj0a, dj0b, dr1]
                    deferred = [dj1a, dj1b] + [
                        lambda tb=tb: outproj(tb)
                        for tb in range(4 * tt, 4 * tt + 4)]
                # flush (the last tile's deferred work included)
                queue += deferred
                pop(len(queue))

    if split:
        _split_waits(nc)
    return nc


def make_in_maps(x, rope_cache, Wqkv, bqkv, Wout, bout):
    """Host-side shard prep. Returns list of 8 in_maps (core = 4*b + g)."""
    x = np.asarray(x, np.float32)
    rope_cache = np.asarray(rope_cache, np.float32)
    Wqkv = np.asarray(Wqkv, np.float32)
    bqkv = np.asarray(bqkv, np.float32)
    Wout = np.asarray(Wout, np.float32)

    # rotary-half permutation within a head: [evens, odds]
    perm = np.concatenate([np.arange(0, D, 2), np.arange(1, D, 2)])
    sin = rope_cache[:, 0::2].T.copy()   # [32, T]
    cos = rope_cache[:, 1::2].T.copy()
    scs = np.concatenate([np.tile(sin, (4, 1)), np.tile(cos, (4, 1))],
                         axis=1).astype(BF)  # [128, 2T]

    xT = [np.ascontiguousarray(x[b].T.astype(BF)) for b in range(B)]

    in_maps = []
    for core in range(N_CORES):
        b, g = divmod(core, G)
        heads = range(HPC * g, HPC * g + HPC)
        # A-block: low halves (even dims) of the 4 heads; B-block: high halves
        qcols, kcols, vcols = [], [], []
        for part in range(2):  # lo, hi
            for h in heads:
                dd = h * D + perm[part * 32:(part + 1) * 32]
                qcols.extend(0 * C + dd)
                kcols.extend(1 * C + dd)
        for h in heads:
            vcols.extend(2 * C + h * D + np.arange(D))
        qcols = np.asarray(qcols)
        kcols = np.asarray(kcols)
        vcols = np.asarray(vcols)
        wq_c = np.ascontiguousarray(Wqkv[:, qcols].astype(BF))
        wk_c = np.ascontiguousarray(Wqkv[:, kcols].astype(BF))
        wv_c = np.zeros((C, 260), np.float32)
        vv = Wqkv[:, vcols]
        for h in range(HPC):
            wv_c[:, 65 * h:65 * h + 64] = vv[:, 64 * h:64 * h + 64]
        bqk_c = np.stack([bqkv[qcols[:128]], bqkv[qcols[128:]],
                          bqkv[kcols[:128]], bqkv[kcols[128:]]], axis=1)
        rows = np.arange(HPC * g * D, (HPC * g + HPC) * D)
        wout_c = np.ascontiguousarray(Wout[rows, :].astype(BF))
        in_maps.append({
            "xT": xT[b], "wq": wq_c, "wk": wk_c,
            "wv": np.ascontiguousarray(wv_c.astype(BF)),
            "bqk": np.ascontiguousarray(bqk_c.astype(np.float32)),
            "scs": scs, "wout": wout_c,
        })
    return in_maps


_NC_CACHE = None


def _get_nc():
    global _NC_CACHE
    if _NC_CACHE is None:
        _NC_CACHE = build_nc()
    return _NC_CACHE


def run(inputs, trace=False):
    nc = _get_nc()
    in_maps = make_in_maps(**inputs)
    res = run_bass_kernel_spmd(nc, in_maps, list(range(N_CORES)), trace=trace)
    bqkv = np.asarray(inputs["bqkv"], np.float32)
    Wout = np.asarray(inputs["Wout"], np.float32)
    bout = np.asarray(inputs["bout"], np.float32)
    # bv commutes through the softmax-weighted average (weights sum to 1):
    # each head's output gains +bv_head, so the final bias is bv @ Wout + bout.
    ybias = bqkv[2 * C:] @ Wout + bout
    out = np.zeros((B, T, C), np.float32)
    for core in range(N_CORES):
        out[core // G] += res.results[core]["y"]
    out += ybias[None, None, :]
    return out, res


def kernel(**inputs):
    out, _ = run(inputs)
    return out


# revision 31
# speedup vs baseline: 1.3301x; 1.0115x over previous
"""Causal self-attention (B=2, T=2048, C=1024, H=16, D=64) on 8 NeuronCores.

Sharding: core = (batch b, head-group g); each of the 8 cores handles one
batch and 4 of the 16 heads (data parallel on B, tensor parallel on heads).
Each core computes q/k/v projections for its heads, rope, causal softmax
attention, and a partial out-projection; the host sums the 4 per-batch
partials and adds bout + bv@Wout (the V-bias commutes through the softmax
because the attention weights sum to 1).

Device dataflow (per core), matmul operands in bf16 (PSUM accumulates fp32;
bf16 halves DMA, enables fast weight loads, and keeps the PE at full clock):
  - host passes x[b].T in bf16 so the contraction dim (C) lands on partitions
  - q,k are produced directly in [dim, t] layout ("A"=low rotary halves of
    4 heads stacked, "B"=high halves); bias-add on DVE, rope on DVE/GpSimd
    with fp32 intermediates
  - S^T = K~^T Q~ per 128k x 512q block, 4 heads packed into the PE array
    via tile_position row groups (contraction=32 each for A/B parts)
  - softmax without max-subtraction (logits are O(1) for this model):
    exp on ScalarE with the 1/8 scale folded in, bf16 output; causal
    masking by multiplying diagonal blocks with constant 0/1 masks
  - O^T = V_aug^T expS^T accumulated over k blocks, where V_aug carries a
    ones column so row 64 of the PSUM accumulator is the softmax
    denominator; divide via DVE reciprocal + K=1 PE broadcast of the recip
    (the DVE multiply reads the broadcast straight from PSUM)
  - partial out-projection [t,c] = (O^T)^T Wout_rows, DMA'd out in fp32
"""
import sys
sys.path.insert(0, '/opt/trn_rl_repo')

import numpy as np
import ml_dtypes
from contextlib import ExitStack

import concourse.bass as bass
import concourse.tile as tile
from concourse import mybir
from concourse.bass_utils import run_bass_kernel_spmd

B, T, C, H, D = 2, 2048, 1024, 16, 64
HPC = 4          # heads per core
G = H // HPC     # head groups (cores per batch)
N_CORES = B * G
SCALE = 1.0 / np.sqrt(D)
P = 128
QT = 512         # q tile width
TT = T // QT     # q tiles
NKB = T // P     # 128-wide k blocks
NTB = T // P     # 128-wide t blocks
NCC = C // P     # 128-deep contraction chunks
F32 = mybir.dt.float32
F32R = mybir.dt.float32r
BF16 = mybir.dt.bfloat16
BF = ml_dtypes.bfloat16


def _tril_mask():
    p = np.arange(P)[:, None]
    f = np.arange(P)[None, :]
    return (p <= f).astype(np.float32)


# walrus in this toolchain can't encode >1 sem wait on one instruction
# ("Too many sync wait commands"); split excess waits onto preceding NoOps.
def _split_waits(nc, maxw=1):
    for f in nc.m.functions:
        for bb in f.blocks:
            out = []
            for inst in bb.instructions:
                si = getattr(inst, 'sync_info', None)
                if si is not None and si.on_wait and len(si.on_wait) > maxw:
                    waits = list(si.on_wait)
                    extra, keep = waits[:-maxw], waits[-maxw:]
                    for i in range(0, len(extra), maxw):
                        out.append(mybir.InstNoOp(
                            name=f"{inst.name}-wsplit{i}",
                            sync_info=mybir.SyncInfo(
                                on_wait=extra[i:i + maxw], on_update=[]),
                            bass_nofuse=True,
                            engine=inst.engine,
                        ))
                    inst.sync_info = mybir.SyncInfo(
                        on_wait=keep, on_update=list(si.on_update or []))
                out.append(inst)
            bb.instructions[:] = out


def build_nc(split=True):
    nc = bass.Bass()
    xT = nc.dram_tensor("xT", [C, T], BF16, kind="ExternalInput")
    wq = nc.dram_tensor("wq", [C, 256], BF16, kind="ExternalInput")
    wk = nc.dram_tensor("wk", [C, 256], BF16, kind="ExternalInput")
    wv = nc.dram_tensor("wv", [C, 260], BF16, kind="ExternalInput")
    bqk = nc.dram_tensor("bqk", [P, 4], F32, kind="ExternalInput")  # qA qB kA kB
    scs = nc.dram_tensor("scs", [P, 2 * T], BF16, kind="ExternalInput")  # sin|cos
    wout = nc.dram_tensor("wout", [256, C], BF16, kind="ExternalInput")
    y = nc.dram_tensor("y", [T, C], F32, kind="ExternalOutput")
    masks_d = nc.inline_tensor(_tril_mask(), name="cmasks")

    with tile.TileContext(nc) as tc:
        with ExitStack() as ctx:
            # ---- resident pools ----
            wpool = ctx.enter_context(tc.tile_pool(name="wts", bufs=1))
            qkpool = ctx.enter_context(tc.tile_pool(name="qk", bufs=1))
            vpool = ctx.enter_context(tc.tile_pool(name="v", bufs=1))
            otpool = ctx.enter_context(tc.tile_pool(name="ot", bufs=1))

            bqk_sb = wpool.tile([P, 4], F32, tag="bqk")
            nc.sync.dma_start(bqk_sb[:], bqk[:])
            wq_sb = wpool.tile([P, NCC, 256], BF16, tag="wq")
            nc.sync.dma_start(wq_sb[:], wq.rearrange("(o p) n -> p o n", p=P))
            wk_sb = wpool.tile([P, NCC, 256], BF16, tag="wk")
            wv_sb = wpool.tile([P, NCC, 260], BF16, tag="wv")
            scs_sb = wpool.tile([P, 2 * T], BF16, tag="scs")
            sin4 = scs_sb[:, 0:T]
            cos4 = scs_sb[:, T:2 * T]
            wout_sb = wpool.tile([P, 2, C], BF16, tag="wout")
            masks_f = wpool.tile([P, P], F32, tag="masksf")
            masks_sb = wpool.tile([P, P], BF16, tag="masks")
            ones_f = wpool.tile([P, 64], F32, tag="onesf")
            nc.vector.memset(ones_f[:], 1.0)
            ones_sb = wpool.tile([P, 64], F32R, tag="ones")
            nc.vector.tensor_copy(ones_sb[:], ones_f[:])

            # q/k in rotary-half layout: A = low halves of 4 heads, B = high
            qA = qkpool.tile([P, T], BF16, tag="qA")
            qB = qkpool.tile([P, T], BF16, tag="qB")
            kA = qkpool.tile([P, T], BF16, tag="kA")
            kB = qkpool.tile([P, T], BF16, tag="kB")
            qk_tiles = [qA, qB, kA, kB]
            w_of = {0: wq_sb, 1: wq_sb, 2: wk_sb, 3: wk_sb}
            col_of = {0: 0, 1: 128, 2: 0, 3: 128}

            # V tiles [t-block, 4*65] (65th col per head becomes ones)
            v_tiles = [vpool.tile([P, 260], BF16, tag=f"v{tb}", name=f"v{tb}")
                       for tb in range(NTB)]

            # O^T: heads 0,1 stacked / heads 2,3 stacked (divided, bf16)
            ot_sb = [otpool.tile([P, T], BF16, tag=f"otsb{i}", name=f"otsb{i}")
                     for i in range(2)]

            with ExitStack() as stream:
                xpool = stream.enter_context(tc.tile_pool(name="x", bufs=2))
                rtmp = stream.enter_context(tc.tile_pool(name="rtmp", bufs=4))
                ps_s = stream.enter_context(
                    tc.tile_pool(name="pss", bufs=2, space="PSUM"))
                ps_ot = stream.enter_context(
                    tc.tile_pool(name="psot", bufs=1, space="PSUM"))
                ps_inj = stream.enter_context(
                    tc.tile_pool(name="psinj", bufs=2, space="PSUM"))
                espool = stream.enter_context(tc.tile_pool(name="es", bufs=44))
                dpool = stream.enter_context(tc.tile_pool(name="dv", bufs=3))
                opool = stream.enter_context(tc.tile_pool(name="osb", bufs=2))

                # one-time mask convert fp32 -> bf16
                nc.sync.dma_start(masks_f[:], masks_d[:])
                nc.vector.tensor_copy(masks_sb[:], masks_f[:])

                xt = {}

                def load_xt(tt):
                    for cc in range(NCC):
                        t = xpool.tile([P, QT], BF16, tag=f"x{cc}",
                                       name=f"x{cc}_{tt}")
                        nc.sync.dma_start(
                            t[:], xT[cc * P:(cc + 1) * P,
                                     tt * QT:(tt + 1) * QT])
                        xt[(cc, tt)] = t

                # q/k projection for one (jb, tt): one 8-chunk PSUM
                # generation, bias-add on DVE straight out of PSUM.
                def qkproj(jb, tt):
                    wsb, c0 = w_of[jb], col_of[jb]
                    dst = qk_tiles[jb][:, tt * QT:(tt + 1) * QT]
                    ps = ps_inj.tile([P, QT], F32, tag="inj",
                                     name="qk_ps")
                    for cc in range(NCC):
                        nc.tensor.matmul(
                            ps, wsb[:, cc, c0:c0 + 128], xt[(cc, tt)][:],
                            start=(cc == 0), stop=(cc == NCC - 1))
                    nc.vector.tensor_scalar_add(dst, ps, bqk_sb[:, jb:jb + 1])

                def rope(pair, tt):
                    At, Bt = qk_tiles[2 * pair], qk_tiles[2 * pair + 1]
                    s = slice(tt * QT, (tt + 1) * QT)
                    t1 = rtmp.tile([P, QT], F32, tag="rt", name="rt1")
                    t2 = rtmp.tile([P, QT], F32, tag="rt", name="rt2")
                    t3 = rtmp.tile([P, QT], F32, tag="rt", name="rt3")
                    nc.gpsimd.tensor_mul(t1[:], At[:, s], cos4[:, s])
                    nc.gpsimd.tensor_mul(t2[:], Bt[:, s], sin4[:, s])
                    nc.vector.tensor_mul(t3[:], At[:, s], sin4[:, s])
                    nc.vector.tensor_sub(At[:, s], t1[:], t2[:])
                    nc.gpsimd.tensor_mul(Bt[:, s], Bt[:, s], cos4[:, s])
                    nc.gpsimd.tensor_add(Bt[:, s], Bt[:, s], t3[:])

                def vproj(tb):
                    ps = ps_inj.tile([P, QT], F32, tag="inj",
                                     name="psv")[:, 0:260]
                    for cc in range(NCC):
                        nc.tensor.matmul(
                            ps, xt[(cc, tb // 4)][:, (tb % 4) * P:
                                                  (tb % 4 + 1) * P],
                            wv_sb[:, cc, :],
                            start=(cc == 0), stop=(cc == NCC - 1))
                    nc.scalar.activation(
                        v_tiles[tb][:], ps,
                        mybir.ActivationFunctionType.Copy)
                    ones_cols = v_tiles[tb].rearrange(
                        "p (h e) -> p h e", e=65)[:, :, 64]
                    nc.gpsimd.tensor_scalar_add(ones_cols, ones_cols, 1.0)

                def drain(acc):
                    # copy one (tt, pp) O^T accumulator out of PSUM and set
                    # up the reciprocal row via a partition-spread DMA round
                    # trip (the consumers pop several rounds later, hiding
                    # the DMA latency); frees the psot slot.
                    otf2 = dpool.tile([65, 2, QT], F32, tag="otf",
                                      name="otf")
                    nc.vector.tensor_copy(otf2[:], acc[:])
                    dn = dpool.tile([P, 8], F32, tag="dn", name="dn")
                    nc.sync.dma_start(
                        dn[:], otf2[64:65, :, :].rearrange(
                            "a b c -> a (b c)"))
                    nc.vector.reciprocal(dn[:], dn[:])
                    rr = dpool.tile([1, 2, QT], F32R, tag="rr", name="rr")
                    nc.sync.dma_start(
                        rr[0:1, :, :].rearrange("a b c -> a (b c)"),
                        dn[:].bitcast(F32R))
                    return (otf2, rr)

                def divide_j(tt, pp, pend, j):
                    otf2, rr = pend
                    rb = ps_inj.tile([P, QT], F32, tag="inj",
                                     name="rb")[0:64, :]
                    nc.tensor.matmul(
                        rb, ones_sb[0:1, 0:64],
                        rr[0:1, j, :], start=True, stop=True)
                    dst = ot_sb[pp][64 * j:64 * j + 64,
                                    tt * QT:(tt + 1) * QT]
                    nc.vector.tensor_mul(dst, otf2[0:64, j, :], rb)

                def outproj(tb):
                    o_sb = opool.tile([P, C], F32, tag="osb", name="osb")
                    for nt in range(2):
                        ps = ps_inj.tile([P, QT], F32, tag="inj",
                                          name="pso")
                        for rc in range(2):
                            nc.tensor.matmul(
                                ps, ot_sb[rc][:, tb * P:(tb + 1) * P],
                                wout_sb[:, rc, nt * 512:(nt + 1) * 512],
                                start=(rc == 0), stop=(rc == 1))
                        nc.vector.tensor_copy(
                            o_sb[:, nt * 512:(nt + 1) * 512], ps)
                    nc.gpsimd.dma_start(y[tb * P:(tb + 1) * P, :], o_sb[:])

                # ---- prologue: tile 0's inputs and projections ----
                t0 = 0
                load_xt(t0)
                nc.gpsimd.dma_start(wk_sb[:],
                                    wk.rearrange("(o p) n -> p o n", p=P))
                nc.scalar.dma_start(scs_sb[:], scs[:])
                nc.gpsimd.dma_start(wv_sb[:],
                                    wv.rearrange("(o p) n -> p o n", p=P))
                nc.scalar.dma_start(wout_sb[:],
                                    wout.rearrange("(o p) n -> p o n", p=P))
                for jb in range(4):
                    qkproj(jb, t0)
                rope(0, t0)
                rope(1, t0)
                for tb in range(4 * t0, 4 * t0 + 4):
                    vproj(tb)

                # ---- streaming attention with a deferred work queue ----
                # The S^T/exp rounds run on a dedicated PSUM ring and are
                # never blocked by other work.  Everything else — the O^T
                # accumulation (one (tt, pp) phase at a time through the
                # single psot slot), its drain/divide, the next tile's
                # projections, and the out-projection — flows through a FIFO
                # of closures popped a few per round.
                queue = []

                def ot_unit(tt, pp, kblk, es2, nk):
                    off = max(0, (kblk - 4 * tt)) * P

                    def go(acc=None):
                        for j in range(2):
                            h = 2 * pp + j
                            nc.tensor.matmul(
                                acc[:, j, off:],
                                v_tiles[kblk][:, 65 * h:65 * h + 65],
                                es2[:, j, off:],
                                start=(kblk == 0), stop=(kblk == nk - 1))
                    return go

                def pop(n):
                    for _ in range(min(n, len(queue))):
                        queue.pop(0)()

                # phase state: the psot slot cycles through (tt, pp) phases;
                # each phase's accumulator is allocated lazily by its first
                # popped unit (FIFO order sequences it after the previous
                # phase's drain).
                def phase_start(tt, pp, es2, nk, box):
                    def first():
                        box['acc'] = ps_ot.tile([65, 2, QT], F32, tag="psot",
                                                name=f"ps_{tt}_{pp}")
                        ot_unit(tt, pp, 0, es2, nk)(box['acc'])
                    return first

                def phase_end(tt, pp, box):
                    def dr():
                        box['pend'] = drain(box['acc'])
                    return ([dr] +
                            [(lambda j=j: divide_j(tt, pp, box['pend'], j))
                             for j in range(2)])

                es_hist = {}
                deferred = []
                order = list(range(TT))
                rounds_after = [sum(4 * o + 4 for o in order[i:])
                                for i in range(len(order))]
                for oi, tt in enumerate(order):
                    nk = 4 * tt + 4
                    box0, box1 = {}, {}
                    if oi + 1 < len(order):
                        ntt = order[oi + 1]
                        load_xt(ntt)
                        # prep for the NEXT tile goes to the queue FRONT —
                        # it's independent of the backlog and must be fully
                        # emitted before that tile's rounds (deps are
                        # emission-order-based)
                        prep = [lambda jb=jb, t=ntt: qkproj(jb, t)
                                for jb in range(4)]
                        prep += [lambda t=ntt: rope(0, t),
                                 lambda t=ntt: rope(1, t)]
                        prep += [lambda tb=tb: vproj(tb)
                                 for tb in range(4 * ntt, 4 * ntt + 4)]
                        queue[0:0] = prep
                    queue += deferred
                    deferred = []
                    rounds_rem_all = rounds_after[oi]
                    for kblk in range(nk):
                        off = max(0, (kblk - 4 * tt)) * P
                        ks = slice(kblk * P, (kblk + 1) * P)
                        qs = slice(tt * QT + off, (tt + 1) * QT)
                        # both pps' S^T tiles up front; all four A-half
                        # matmuls back-to-back (4 distinct PE row groups run
                        # concurrently), then the four B-halves.
                        s2s = [ps_s.tile([P, 2, QT], F32, tag="pss",
                                         name="pss") for _ in range(2)]
                        for half, (t_a, t_b) in enumerate(
                                [(kA, qA), (kB, qB)]):
                            for pp in range(2):
                                for j in range(2):
                                    h = 2 * pp + j
                                    hp = slice(32 * h, 32 * h + 32)
                                    nc.tensor.matmul(
                                        s2s[pp][:, j, off:],
                                        t_a[hp, ks], t_b[hp, qs],
                                        start=(half == 0), stop=(half == 1),
                                        tile_position=(32 * h, 0))
                        for pp in range(2):
                            es2 = espool.tile([P, 2, QT], BF16, tag="es",
                                              name="es")
                            nc.scalar.activation(
                                es2[:, :, off:], s2s[pp][:, :, off:],
                                mybir.ActivationFunctionType.Exp, scale=SCALE)
                            if kblk >= 4 * tt:
                                nc.vector.tensor_mul(
                                    es2[:, :, off:off + P],
                                    es2[:, :, off:off + P],
                                    masks_sb[:, None, :].to_broadcast(
                                        (P, 2, P)))
                            es_hist[(tt, kblk, pp)] = es2
                            # queue pp0's O^T for this block as soon as its
                            # es exists; pp1's phase runs after pp0 drains.
                            if pp == 0:
                                if kblk == 0:
                                    queue.append(
                                        phase_start(tt, 0, es2, nk, box0))
                                else:
                                    queue.append(
                                        lambda k=kblk, e=es2, t=tt, n=nk,
                                        b=box0:
                                        ot_unit(t, 0, k, e, n)(b['acc']))
                        # steady drain: pace the backlog to finish by the
                        # final round overall; at least 2 per round
                        rem = rounds_rem_all - kblk - 1
                        npop = (-(-len(queue) // rem) if rem > 0
                                else len(queue))
                        pop(max(npop, 2))
                    # pp0's drain, then the whole pp1 phase, THEN pp0's
                    # divides (spaced nk units after their drain so the
                    # denominator DMA round-trip is hidden), then pp1's
                    # drain; pp1's divides and the outproj are deferred to
                    # the next tile's queue (spaced behind its projections).
                    dr0, dj0a, dj0b = phase_end(tt, 0, box0)
                    queue.append(dr0)
                    queue.append(phase_start(tt, 1, es_hist[(tt, 0, 1)],
                                             nk, box1))
                    queue += [lambda k=k, e=es_hist[(tt, k, 1)], t=tt, n=nk,
                              b=box1: ot_unit(t, 1, k, e, n)(b['acc'])
                              for k in range(1, nk)]
                    dr1, dj1a, dj1b = phase_end(tt, 1, box1)
                    queue += [dj0a, dj0b, dr1]
                    deferred = [dj1a, dj1b] + [
                        lambda tb=tb: outproj(tb)
                        for tb in range(4 * tt, 4 * tt + 4)]
                # flush (the last tile's deferred work included)
                queue += deferred
                pop(len(queue))

    if split:
        _split_waits(nc)
    return nc


def make_in_maps(x, rope_cache, Wqkv, bqkv, Wout, bout):
    """Host-side shard prep. Returns list of 8 in_maps (core = 4*b + g)."""
    x = np.asarray(x, np.float32)
    rope_cache = np.asarray(rope_cache, np.float32)
    Wqkv = np.asarray(Wqkv, np.float32)
    bqkv = np.asarray(bqkv, np.float32)
    Wout = np.asarray(Wout, np.float32)

    # rotary-half permutation within a head: [evens, odds]
    perm = np.concatenate([np.arange(0, D, 2), np.arange(1, D, 2)])
    sin = rope_cache[:, 0::2].T.copy()   # [32, T]
    cos = rope_cache[:, 1::2].T.copy()
    scs = np.concatenate([np.tile(sin, (4, 1)), np.tile(cos, (4, 1))],
                         axis=1).astype(BF)  # [128, 2T]

    xT = [np.ascontiguousarray(x[b].T.astype(BF)) for b in range(B)]

    in_maps = []
    for core in range(N_CORES):
        b, g = divmod(core, G)
        heads = range(HPC * g, HPC * g + HPC)
        # A-block: low halves (even dims) of the 4 heads; B-block: high halves
        qcols, kcols, vcols = [], [], []
        for part in range(2):  # lo, hi
            for h in heads:
                dd = h * D + perm[part * 32:(part + 1) * 32]
                qcols.extend(0 * C + dd)
                kcols.extend(1 * C + dd)
        for h in heads:
            vcols.extend(2 * C + h * D + np.arange(D))
        qcols = np.asarray(qcols)
        kcols = np.asarray(kcols)
        vcols = np.asarray(vcols)
        wq_c = np.ascontiguousarray(Wqkv[:, qcols].astype(BF))
        wk_c = np.ascontiguousarray(Wqkv[:, kcols].astype(BF))
        wv_c = np.zeros((C, 260), np.float32)
        vv = Wqkv[:, vcols]
        for h in range(HPC):
            wv_c[:, 65 * h:65 * h + 64] = vv[:, 64 * h:64 * h + 64]
        bqk_c = np.stack([bqkv[qcols[:128]], bqkv[qcols[128:]],
                          bqkv[kcols[:128]], bqkv[kcols[128:]]], axis=1)
        rows = np.arange(HPC * g * D, (HPC * g + HPC) * D)
        wout_c = np.ascontiguousarray(Wout[rows, :].astype(BF))
        in_maps.append({
            "xT": xT[b], "wq": wq_c, "wk": wk_c,
            "wv": np.ascontiguousarray(wv_c.astype(BF)),
            "bqk": np.ascontiguousarray(bqk_c.astype(np.float32)),
            "scs": scs, "wout": wout_c,
        })
    return in_maps


_NC_CACHE = None


def _get_nc():
    global _NC_CACHE
    if _NC_CACHE is None:
        _NC_CACHE = build_nc()
    return _NC_CACHE


def run(inputs, trace=False):
    nc = _get_nc()
    in_maps = make_in_maps(**inputs)
    res = run_bass_kernel_spmd(nc, in_maps, list(range(N_CORES)), trace=trace)
    bqkv = np.asarray(inputs["bqkv"], np.float32)
    Wout = np.asarray(inputs["Wout"], np.float32)
    bout = np.asarray(inputs["bout"], np.float32)
    # bv commutes through the softmax-weighted average (weights sum to 1):
    # each head's output gains +bv_head, so the final bias is bv @ Wout + bout.
    ybias = bqkv[2 * C:] @ Wout + bout
    out = np.zeros((B, T, C), np.float32)
    for core in range(N_CORES):
        out[core // G] += res.results[core]["y"]
    out += ybias[None, None, :]
    return out, res


def kernel(**inputs):
    out, _ = run(inputs)
    return out
